# revision 19
# baseline (speedup 1.0000x reference)
"""EventSpecificTimingHeads Trainium2 kernel (8 NeuronCores, SPMD).

Shards the E=16 independent per-event attention+MLP heads across 8 cores
(2 events per core). Each core computes logits[e, b, s] for its 2 events
over the full shared feature tensor; the host gathers and transposes to
[B, S, E].

Per (event, batch), in the transposed (j, i) orientation:
  scores.T = k q.T per 32-row head group (4-way PE row tiling)
  P.T = exp(scores.T), split between ACT (exact) and DVE (Schraudolph
    fast exp: int16(x*128/ln2 + B) bitcast to bf16; ~3% per element,
    self-cancelling through softmax normalization)
  pv.T = v_aug.T @ P.T, v_aug = [v | ones | zero-pad] (M=64, two heads
    per 128x512 psum half at column positions 0/64) -> ctx rows plus the
    softmax denominators l as rows 32/96, all in the same streams
  normalize: reciprocal_approx_fast on the strided l rows -> bf16 cast
    -> K=2 select-matmul broadcasts 1/l across each 64-row block ->
    one tensor_tensor multiply per half (both operands PSUM)
  Wo is folded into W1 on the host; W1'' is permuted/zero-padded to
    read the packed ctx layout via two accumulating matmuls
  h1 = relu(W1'' ctx + c1); logits = w2.T h1 + b2 (M=32 zero-padded,
    two (ev,b) per psum tile)
The loop is software-pipelined: iteration `it` emits QK+exp(it) but
PV/normalize/W1(it-1) and the MLP tail of the pair ending at it-3, so
every PE instruction's dependencies are one iteration stale and the PE
never idles (keeps the tensor engine at its ramped clock).
"""
import sys

if "/opt/trn_rl_repo" not in sys.path:
    sys.path.insert(0, "/opt/trn_rl_repo")

import numpy as np
import ml_dtypes

import concourse.bass as bass
import concourse.bacc as bacc
import concourse.tile as tile
from concourse import mybir
from concourse.bass_utils import run_bass_kernel_spmd

BF16 = mybir.dt.bfloat16
F32 = mybir.dt.float32
I16 = mybir.dt.int16
AF = mybir.ActivationFunctionType
ALU = mybir.AluOpType

E, D, B, S, H, Dh, H2 = 16, 128, 8, 512, 4, 32, 64
T = B * S            # 4096
EV = 2               # events per core
NCORES = 8

# Schraudolph fast-exp constants (bf16 target, trunc-compensated)
EXP_A = 128.0 / float(np.log(2.0))     # 184.664965
EXP_B = 16256.0 - 5.59 + 0.5

# Per-(ev,b) exp engine split: tile k (of 8 [128,1024] tiles) -> DVE if in set
DVE_TILES_EVEN = (2, 6)
DVE_TILES_ODD = (2, 5, 7)

_CACHED_NC = None


def build_nc():
    nc = bacc.Bacc(None, target_bir_lowering=False, debug=False)

    xT_d = nc.declare_dram_parameter("xT", [D, T], BF16, isOutput=False)
    wqkT_d = nc.declare_dram_parameter("wqkT", [D, EV, 2, D], BF16, isOutput=False)
    wvT_d = nc.declare_dram_parameter("wvT", [D, EV, D], BF16, isOutput=False)
    bqkT_d = nc.declare_dram_parameter("bqkT", [1, EV, 2, D], BF16, isOutput=False)
    w1pT_d = nc.declare_dram_parameter("w1pT", [D, EV, 2, H2], BF16, isOutput=False)
    c1b2_d = nc.declare_dram_parameter("c1b2", [D, EV], F32, isOutput=False)
    w2dup_d = nc.declare_dram_parameter("w2dup", [D, EV, 32], BF16, isOutput=False)
    b2rep_d = nc.declare_dram_parameter("b2rep", [D, EV], F32, isOutput=False)
    out_d = nc.declare_dram_parameter("out", [EV, B, S], F32, isOutput=True)

    with tile.TileContext(nc) as tc:
        with (
            tc.tile_pool(name="single", bufs=1) as single,
            tc.tile_pool(name="work", bufs=2) as work,
            tc.tile_pool(name="ps", bufs=1, space="PSUM") as psp,
        ):
            # ---- resident SBUF tensors ----
            xT_sb = single.tile([D, T], BF16)
            wqkT_sb = single.tile([D, EV, 2, D], BF16)
            wvT_sb = single.tile([D, EV, D], BF16)
            bqkT_sb = single.tile([1, EV, 2, D], BF16)
            w1pT_sb = single.tile([D, EV, 2, H2], BF16)
            c1b2_sb = single.tile([D, EV], F32)
            w2dup_sb = single.tile([D, EV, 32], BF16)
            b2rep_sb = single.tile([D, EV], F32)
            ones1 = single.tile([1, S], BF16)
            onesW = single.tile([D, 64], BF16)
            # q/k (bf16, bias folded in): [d, ev, b, {q,k}, s-in-chunk]
            qkT_sb = single.tile([D, EV, B, 2, S], BF16)
            # v_aug: [j-in-chunk, b, jc, ev, h, 64]; col 32 ones, 33:64 zero
            v_sb = single.tile([D, B, 4, EV, H, 64], BF16)

            nc.sync.dma_start(out=wqkT_sb[:], in_=wqkT_d[:])
            nc.sync.dma_start(out=bqkT_sb[:], in_=bqkT_d[:])
            nc.sync.dma_start(out=wvT_sb[:], in_=wvT_d[:])
            for n in range(8):
                nc.scalar.dma_start(out=xT_sb[:, n * S:(n + 1) * S],
                                    in_=xT_d[:, n * S:(n + 1) * S])
            nc.sync.dma_start(out=w1pT_sb[:], in_=w1pT_d[:])
            nc.sync.dma_start(out=c1b2_sb[:], in_=c1b2_d[:])
            nc.sync.dma_start(out=w2dup_sb[:], in_=w2dup_d[:])
            nc.sync.dma_start(out=b2rep_sb[:], in_=b2rep_d[:])
            nc.gpsimd.memset(ones1[:], 1.0)
            nc.gpsimd.memset(onesW[:], 1.0)
            nc.gpsimd.memset(v_sb[:, :, :, :, :, 32:33], 1.0)
            nc.gpsimd.memset(v_sb[:, :, :, :, :, 33:64], 0.0)

            def proj_qk(eb):
                """q,k projection for one (ev, chunk b); bias via K=1 matmul,
                drain is a pure copy (engine alternates by eb parity)."""
                ev, b = eb // B, eb % B
                t0 = b * S
                ps = psp.tile([D, 2, S], F32, name="proj", tag="st", bufs=2)
                for qk in range(2):
                    nc.tensor.matmul(
                        ps[:, qk, :],
                        wqkT_sb[:, ev, qk, :],
                        xT_sb[:, t0:t0 + S],
                        start=True, stop=False,
                    )
                    nc.tensor.matmul(
                        ps[:, qk, :],
                        bqkT_sb[0:1, ev, qk, :],
                        ones1[:],
                        start=False, stop=True,
                    )
                if eb % 2 == 0:
                    nc.vector.tensor_copy(qkT_sb[:, ev, b, :, :], ps[:])
                else:
                    nc.scalar.activation(qkT_sb[:, ev, b, :, :], ps[:], AF.Copy)

            def project_v(b):
                """v for both events of chunk-group b; pure-copy drain."""
                psv = psp.tile([D, 2, S], F32, name="psv", tag="st", bufs=2)
                for c in range(4):
                    tch = 4 * b + c
                    nc.tensor.matmul(
                        psv[:, c // 2, (c % 2) * 256:(c % 2) * 256 + 256],
                        xT_sb[:, tch * D:(tch + 1) * D],
                        wvT_sb[:].rearrange("p e d -> p (e d)"),
                    )
                nc.vector.tensor_copy(
                    v_sb[:, b, :, :, :, 0:32],
                    psv[:].rearrange("p a (f e h d) -> p (a f) e h d",
                                     f=2, e=EV, h=H),
                )

            pts = {}
            mlps = {}

            def stage_qk(eb):
                """QK^T + exp for (ev, b) = divmod(eb, B)."""
                ev, b = eb // B, eb % B
                pt = work.tile([D, 4, H, S], BF16, name="pt", tag="pt")
                pts[eb] = pt
                dve_tiles = DVE_TILES_EVEN if eb % 2 == 0 else DVE_TILES_ODD
                for k in range(8):
                    jc, hp = k // 2, k % 2
                    st = psp.tile([D, 2, S], F32, name="st", tag="st", bufs=2)
                    for h2 in range(2):
                        h = 2 * hp + h2
                        nc.tensor.matmul(
                            st[:, h2, :],
                            qkT_sb[32 * h:32 * h + 32, ev, b, 1,
                                   jc * D:(jc + 1) * D],
                            qkT_sb[32 * h:32 * h + 32, ev, b, 0, :],
                            tile_position=(32 * h, 0),
                        )
                    dst = pt[:, jc, 2 * hp:2 * hp + 2, :]
                    if k in dve_tiles:
                        nc.vector.tensor_scalar(
                            dst.bitcast(I16), st[:], EXP_A, EXP_B,
                            ALU.mult, ALU.add,
                        )
                    else:
                        nc.scalar.activation(dst, st[:], AF.Exp)

            def stage_pv(eb):
                """PV with augmented v, then normalize -> ctxT2, then W1''."""
                ev, b = eb // B, eb % B
                pt = pts.pop(eb)
                pvab = psp.tile([D, 2, S], F32, name="pvab", tag="pvab")
                for h in range(H):
                    for jc in range(4):
                        nc.tensor.matmul(
                            pvab[64 * (h % 2):64 * (h % 2) + 64, h // 2, :],
                            v_sb[:, b, jc, ev, h, :],
                            pt[:, jc, h, :],
                            start=(jc == 0), stop=(jc == 3),
                            tile_position=(0, 64 * (h % 2)),
                        )
                lrow = work.tile([D, 2, S], BF16, name="lrow", tag="lrow")
                nc.vector.tensor_copy(lrow[32:33], pvab[32:33, :, :])
                nc.scalar.activation(lrow[96:97], pvab[96:97, :, :], AF.Copy)
                lsumb = psp.tile([D, 2, S], F32, name="lsumb", tag="st",
                                 bufs=2)
                for g in range(2):
                    nc.tensor.matmul(
                        lsumb[0:64, g, :], onesW[32:33, :],
                        lrow[32:33, g, :], tile_position=(32, 0))
                    nc.tensor.matmul(
                        lsumb[64:D, g, :], onesW[96:97, :],
                        lrow[96:97, g, :], tile_position=(96, 64))
                linv = work.tile([D, 2, S], F32, name="linv", tag="linv")
                nc.vector.reciprocal_approx_fast(out=linv[:], in_=lsumb[:])
                ctxT2 = work.tile([D, 2, S], BF16, name="ctxT2", tag="ctxT2")
                mlp = mlps.get(eb // 2)
                if mlp is None:
                    mlp = psp.tile([D, S], F32, name="mlp", tag="mlp")
                    mlps[eb // 2] = mlp
                nc.vector.tensor_tensor(ctxT2[:], pvab[:], linv[:], ALU.mult)
                # W1'' : two accumulating matmuls over the packed ctx halves
                half = 64 * (eb % 2)
                for g in range(2):
                    nc.tensor.matmul(
                        mlp[half:half + 64, :],
                        w1pT_sb[:, ev, g, :],
                        ctxT2[:, g, :],
                        start=(g == 0), stop=(g == 1),
                        tile_position=(0, half),
                    )

            def mlp_tail(p):
                """relu + 2x W2 + logits + DMA for pair p = (2p, 2p+1)."""
                ev = (2 * p) // B
                bb = (2 * p) % B
                mlp = mlps.pop(p)
                h1_sb = work.tile([D, S], BF16, name="h1", tag="h1")
                nc.scalar.activation(
                    h1_sb[:], mlp[:], AF.Relu,
                    bias=c1b2_sb[:, ev:ev + 1],
                )
                w2ps = psp.tile([D, 2, S], F32, name="w2ps", tag="st", bufs=2)
                for j in range(2):
                    hh = 64 * j
                    nc.tensor.matmul(
                        w2ps[32 * j:32 * j + 32, 0, :],
                        w2dup_sb[hh:hh + 64, ev, :],
                        h1_sb[hh:hh + 64, :],
                        tile_position=(hh, 32 * j),
                    )
                lg = work.tile([33, S], F32, name="lg", tag="lg")
                nc.scalar.activation(
                    lg[:], w2ps[0:33, 0, :], AF.Identity,
                    bias=b2rep_sb[0:33, ev:ev + 1],
                )
                nc.sync.dma_start(
                    out=out_d[ev, bb:bb + 2, :],
                    in_=lg[0:33:32, :],
                )

            # ---- software-pipelined main loop ----
            project_v(0)
            proj_qk(0)
            proj_qk(1)
            NEB = EV * B
            for it in range(NEB + 2):
                if it < NEB:
                    stage_qk(it)
                if it >= 3 and (it - 3) % 2 == 0:
                    mlp_tail((it - 3) // 2)
                if 1 <= it <= NEB:
                    stage_pv(it - 1)
                if it + 2 < NEB:
                    proj_qk(it + 2)
                if it < B - 1:
                    project_v(it + 1)

    nc.compile()
    return nc


def _prep_inputs(lstm_features, Wqkv, bqkv, Wo, bo, W1, b1, W2, b2):
    """Host-side per-core input prep (numpy, fp32 -> bf16 where PE-facing)."""
    bf = ml_dtypes.bfloat16
    x = np.asarray(lstm_features, np.float32).reshape(T, D)
    xT = np.ascontiguousarray(x.T).astype(bf)
    scale = 1.0 / np.sqrt(np.float32(Dh))

    in_maps = []
    for c in range(NCORES):
        evs = [2 * c, 2 * c + 1]
        wqkT = np.zeros((D, EV, 2, D), np.float32)
        bqkT = np.zeros((1, EV, 2, D), np.float32)
        wvT = np.zeros((D, EV, D), np.float32)
        w1pT = np.zeros((D, EV, 2, H2), np.float32)
        c1b2 = np.zeros((D, EV), np.float32)
        w2dup = np.zeros((D, EV, 32), np.float32)
        b2rep = np.zeros((D, EV), np.float32)
        for i, e in enumerate(evs):
            Wq = Wqkv[e, 0:D, :] * scale
            Wk = Wqkv[e, D:2 * D, :]
            Wv = Wqkv[e, 2 * D:3 * D, :]
            wqkT[:, i, 0, :] = Wq.T
            wqkT[:, i, 1, :] = Wk.T
            wvT[:, i, :] = Wv.T
            bqkT[0, i, 0, :] = bqkv[e, 0:D] * scale
            bqkT[0, i, 1, :] = bqkv[e, D:2 * D]
            bv = bqkv[e, 2 * D:3 * D]
            bo_eff = Wo[e] @ bv + bo[e]
            W1p = W1[e] @ Wo[e]           # [H2, D]
            # permuted/zero-padded for the packed ctx layout:
            # ctx half g rows 0:32 = head 2g, rows 64:96 = head 2g+1
            for g in range(2):
                w1pT[0:32, i, g, :] = W1p[:, 64 * g:64 * g + 32].T
                w1pT[64:96, i, g, :] = W1p[:, 64 * g + 32:64 * g + 64].T
            c1 = W1[e] @ bo_eff + b1[e]   # [H2]
            c1b2[0:H2, i] = c1
            c1b2[H2:D, i] = c1
            w2dup[0:H2, i, 0] = W2[e, 0, :]
            w2dup[H2:D, i, 0] = W2[e, 0, :]
            b2rep[:, i] = b2[e, 0]
        in_maps.append({
            "xT": xT,
            "wqkT": wqkT.astype(bf),
            "wvT": wvT.astype(bf),
            "bqkT": bqkT.astype(bf),
            "w1pT": w1pT.astype(bf),
            "c1b2": c1b2,
            "w2dup": w2dup.astype(bf),
            "b2rep": b2rep,
        })
    return in_maps


def kernel(lstm_features, Wqkv, bqkv, Wo, bo, W1, b1, W2, b2, _trace=False):
    global _CACHED_NC
    args = [np.asarray(a, np.float32) for a in
            (lstm_features, Wqkv, bqkv, Wo, bo, W1, b1, W2, b2)]
    in_maps = _prep_inputs(*args)
    if _CACHED_NC is None:
        _CACHED_NC = build_nc()
    res = run_bass_kernel_spmd(
        _CACHED_NC, in_maps, list(range(NCORES)), trace=_trace
    )
    logits = np.concatenate(
        [np.asarray(res.results[c]["out"], np.float32) for c in range(NCORES)],
        axis=0,
    )  # [16, 8, 512]
    out = np.ascontiguousarray(logits.transpose(1, 2, 0))  # [B, S, E]
    if _trace:
        return out, res
    return out


# revision 22
# speedup vs baseline: 1.1815x; 1.1815x over previous
"""EventSpecificTimingHeads Trainium2 kernel (8 NeuronCores, SPMD).

Shards the E=16 independent per-event attention+MLP heads across 8 cores
(2 events per core). Each core computes logits[e, b, s] for its 2 events
over the full shared feature tensor; the host gathers and transposes to
[B, S, E].

Per (event, batch), in the transposed (j, i) orientation:
  scores.T = k q.T per 32-row head group (4-way PE row tiling)
  P.T = exp(scores.T), split between ACT (exact) and DVE (Schraudolph
    fast exp: int16(x*128/ln2 + B) bitcast to bf16; ~3% per element,
    self-cancelling through softmax normalization)
  pv.T = v_aug.T @ P.T, v_aug = [v | ones | zero-pad] (M=64, two heads
    per 128x512 psum half at column positions 0/64) -> ctx rows plus the
    softmax denominators l as rows 32/96, all in the same streams
  normalize: reciprocal_approx_fast on the strided l rows -> bf16 cast
    -> K=2 select-matmul broadcasts 1/l across each 64-row block ->
    one tensor_tensor multiply per half (both operands PSUM)
  Wo is folded into W1 on the host; W1'' is permuted/zero-padded to
    read the packed ctx layout via two accumulating matmuls
  h1 = relu(W1'' ctx + c1); logits = w2.T h1 + b2 (M=32 zero-padded,
    two (ev,b) per psum tile)
The loop is software-pipelined: iteration `it` emits QK+exp(it) but
PV/normalize/W1(it-1) and the MLP tail of the pair ending at it-3, so
every PE instruction's dependencies are one iteration stale and the PE
never idles (keeps the tensor engine at its ramped clock).
"""
import sys

if "/opt/trn_rl_repo" not in sys.path:
    sys.path.insert(0, "/opt/trn_rl_repo")

import numpy as np
import ml_dtypes

import concourse.bass as bass
import concourse.bacc as bacc
import concourse.tile as tile
from concourse import mybir
from concourse.bass_utils import run_bass_kernel_spmd

BF16 = mybir.dt.bfloat16
F32 = mybir.dt.float32
I16 = mybir.dt.int16
AF = mybir.ActivationFunctionType
ALU = mybir.AluOpType

E, D, B, S, H, Dh, H2 = 16, 128, 8, 512, 4, 32, 64
T = B * S            # 4096
EV = 2               # events per core
NCORES = 8

# Schraudolph fast-exp constants (bf16 target, trunc-compensated)
EXP_A = 128.0 / float(np.log(2.0))     # 184.664965
EXP_B = 16256.0 - 5.59 + 0.5

# Per-(ev,b) exp engine split: tile k (of 8 [128,1024] tiles) -> DVE if in set
DVE_TILES_EVEN = (2, 6)
DVE_TILES_ODD = (2, 5, 7)

_CACHED_NC = None


def build_nc():
    nc = bacc.Bacc(None, target_bir_lowering=False, debug=False)

    xT_d = nc.declare_dram_parameter("xT", [D, T], BF16, isOutput=False)
    wqkT_d = nc.declare_dram_parameter("wqkT", [D, EV, 2, D], BF16, isOutput=False)
    wvT_d = nc.declare_dram_parameter("wvT", [D, EV, D], BF16, isOutput=False)
    bqk_d = nc.declare_dram_parameter("bqk", [D, EV, 2], F32, isOutput=False)
    w1pT_d = nc.declare_dram_parameter("w1pT", [D, EV, 2, H2], BF16, isOutput=False)
    c1b2_d = nc.declare_dram_parameter("c1b2", [D, EV], F32, isOutput=False)
    w2dup_d = nc.declare_dram_parameter("w2dup", [D, EV, 32], BF16, isOutput=False)
    b2rep_d = nc.declare_dram_parameter("b2rep", [D, EV], F32, isOutput=False)
    out_d = nc.declare_dram_parameter("out", [EV, B, S], F32, isOutput=True)

    with tile.TileContext(nc) as tc:
        with (
            tc.tile_pool(name="single", bufs=1) as single,
            tc.tile_pool(name="work", bufs=2) as work,
            tc.tile_pool(name="ps", bufs=1, space="PSUM") as psp,
        ):
            # ---- resident SBUF tensors ----
            xT_sb = single.tile([D, T], BF16)
            wqkT_sb = single.tile([D, EV, 2, D], BF16)
            wvT_sb = single.tile([D, EV, D], BF16)
            bqk_sb = single.tile([D, EV, 2], F32)
            w1pT_sb = single.tile([D, EV, 2, H2], BF16)
            c1b2_sb = single.tile([D, EV], F32)
            w2dup_sb = single.tile([D, EV, 32], BF16)
            b2rep_sb = single.tile([D, EV], F32)
            onesW = single.tile([D, 64], BF16)
            # q/k (bf16, bias folded in): [d, ev, b, {q,k}, s-in-chunk]
            qkT_sb = single.tile([D, EV, B, 2, S], BF16)
            # v_aug: [j-in-chunk, b, jc, ev, h, 64]; col 32 ones, 33:64 zero
            v_sb = single.tile([D, B, 4, EV, H, 64], BF16)

            nc.sync.dma_start(out=wqkT_sb[:], in_=wqkT_d[:])
            nc.sync.dma_start(out=bqk_sb[:], in_=bqk_d[:])
            nc.sync.dma_start(out=wvT_sb[:], in_=wvT_d[:])
            for n in range(8):
                nc.scalar.dma_start(out=xT_sb[:, n * S:(n + 1) * S],
                                    in_=xT_d[:, n * S:(n + 1) * S])
            nc.sync.dma_start(out=w1pT_sb[:], in_=w1pT_d[:])
            nc.sync.dma_start(out=c1b2_sb[:], in_=c1b2_d[:])
            nc.sync.dma_start(out=w2dup_sb[:], in_=w2dup_d[:])
            nc.sync.dma_start(out=b2rep_sb[:], in_=b2rep_d[:])
            nc.gpsimd.memset(onesW[:], 1.0)
            nc.gpsimd.memset(v_sb[:, :, :, :, :, 32:33], 1.0)
            nc.gpsimd.memset(v_sb[:, :, :, :, :, 33:64], 0.0)

            def proj_qk_mm(eb):
                """q,k projection matmuls for one (ev, chunk b)."""
                ev, b = eb // B, eb % B
                t0 = b * S
                ps = psp.tile([D, 2, S], F32, name="proj", tag="st", bufs=2)
                for qk in range(2):
                    nc.tensor.matmul(
                        ps[:, qk, :],
                        wqkT_sb[:, ev, qk, :],
                        xT_sb[:, t0:t0 + S],
                    )
                return ps

            def proj_qk_drain(eb, ps):
                """bias-fused psum drain (engine alternates by eb parity)."""
                ev, b = eb // B, eb % B
                if eb % 2 == 0:
                    for qk in range(2):
                        nc.vector.tensor_scalar_add(
                            qkT_sb[:, ev, b, qk, :], ps[:, qk, :],
                            bqk_sb[:, ev, qk:qk + 1])
                else:
                    for qk in range(2):
                        nc.scalar.activation(
                            qkT_sb[:, ev, b, qk, :], ps[:, qk, :],
                            AF.Identity, bias=bqk_sb[:, ev, qk:qk + 1])

            def project_v(b):
                """v for both events of chunk-group b; pure-copy drain."""
                psv = psp.tile([D, 2, S], F32, name="psv", tag="st", bufs=2)
                for c in range(4):
                    tch = 4 * b + c
                    nc.tensor.matmul(
                        psv[:, c // 2, (c % 2) * 256:(c % 2) * 256 + 256],
                        xT_sb[:, tch * D:(tch + 1) * D],
                        wvT_sb[:].rearrange("p e d -> p (e d)"),
                    )
                nc.vector.tensor_copy(
                    v_sb[:, b, :, :, :, 0:32],
                    psv[:].rearrange("p a (f e h d) -> p (a f) e h d",
                                     f=2, e=EV, h=H),
                )

            pts = {}
            mlps = {}

            def emit_iteration(it):
                """One pipelined iteration: QK+exp(it) interleaved with
                PV/normalize/W1(it-1) so the PE stream never head-blocks
                on the exp engines for long."""
                cur = it if it < EV * B else None
                prv = it - 1 if 1 <= it <= EV * B else None
                if cur is not None:
                    cev, cb = cur // B, cur % B
                    pt_c = work.tile([D, 4, H, S], BF16, name="pt", tag="pt")
                    pts[cur] = pt_c
                    dve_tiles = (DVE_TILES_EVEN if cur % 2 == 0
                                 else DVE_TILES_ODD)
                if prv is not None:
                    pev, pb = prv // B, prv % B
                    pt_p = pts.pop(prv)
                    pvab = psp.tile([D, 2, S], F32, name="pvab", tag="pvab")

                def qk_tile(k):
                    if cur is None:
                        return
                    jc, hp = k // 2, k % 2
                    st = psp.tile([D, 2, S], F32, name="st", tag="st", bufs=2)
                    for h2 in range(2):
                        h = 2 * hp + h2
                        nc.tensor.matmul(
                            st[:, h2, :],
                            qkT_sb[32 * h:32 * h + 32, cev, cb, 1,
                                   jc * D:(jc + 1) * D],
                            qkT_sb[32 * h:32 * h + 32, cev, cb, 0, :],
                            tile_position=(32 * h, 0),
                        )
                    dst = pt_c[:, jc, 2 * hp:2 * hp + 2, :]
                    if k in dve_tiles:
                        nc.vector.tensor_scalar(
                            dst.bitcast(I16), st[:], EXP_A, EXP_B,
                            ALU.mult, ALU.add,
                        )
                    else:
                        nc.scalar.activation(dst, st[:], AF.Exp)

                def pv_head(h):
                    if prv is None:
                        return
                    for jc in range(4):
                        nc.tensor.matmul(
                            pvab[64 * (h % 2):64 * (h % 2) + 64, h // 2, :],
                            v_sb[:, pb, jc, pev, h, :],
                            pt_p[:, jc, h, :],
                            start=(jc == 0), stop=(jc == 3),
                            tile_position=(0, 64 * (h % 2)),
                        )

                qk_tile(0)
                qk_tile(1)
                pv_head(0)
                qk_tile(2)
                pv_head(1)
                qk_tile(3)
                pv_head(2)
                qk_tile(4)
                pv_head(3)
                if prv is not None:
                    lrow = work.tile([D, 2, S], BF16, name="lrow", tag="lrow")
                    nc.vector.tensor_copy(lrow[32:33], pvab[32:33, :, :])
                    nc.scalar.activation(lrow[96:97], pvab[96:97, :, :],
                                         AF.Copy)
                qk_tile(5)
                if prv is not None:
                    lsumb = psp.tile([D, 2, S], F32, name="lsumb", tag="st",
                                     bufs=2)
                    for g in range(2):
                        nc.tensor.matmul(
                            lsumb[0:64, g, :], onesW[32:33, :],
                            lrow[32:33, g, :], tile_position=(32, 0))
                        nc.tensor.matmul(
                            lsumb[64:D, g, :], onesW[96:97, :],
                            lrow[96:97, g, :], tile_position=(96, 64))
                qk_tile(6)
                if prv is not None:
                    linv = work.tile([D, 2, S], F32, name="linv", tag="linv")
                    nc.vector.reciprocal_approx_fast(out=linv[:],
                                                     in_=lsumb[:])
                    ctxT2 = work.tile([D, 2, S], BF16, name="ctxT2",
                                      tag="ctxT2")
                    nc.vector.tensor_tensor(ctxT2[:], pvab[:], linv[:],
                                            ALU.mult)
                    mlp = mlps.get(prv // 2)
                    if mlp is None:
                        mlp = psp.tile([D, S], F32, name="mlp", tag="mlp")
                        mlps[prv // 2] = mlp
                    half = 64 * (prv % 2)
                    for g in range(2):
                        nc.tensor.matmul(
                            mlp[half:half + 64, :],
                            w1pT_sb[:, pev, g, :],
                            ctxT2[:, g, :],
                            start=(g == 0), stop=(g == 1),
                            tile_position=(0, half),
                        )
                qk_tile(7)

            def mlp_tail(p):
                """relu + 2x W2 + logits + DMA for pair p = (2p, 2p+1)."""
                ev = (2 * p) // B
                bb = (2 * p) % B
                mlp = mlps.pop(p)
                h1_sb = work.tile([D, S], BF16, name="h1", tag="h1")
                nc.scalar.activation(
                    h1_sb[:], mlp[:], AF.Relu,
                    bias=c1b2_sb[:, ev:ev + 1],
                )
                w2ps = psp.tile([D, 2, S], F32, name="w2ps", tag="st", bufs=2)
                for j in range(2):
                    hh = 64 * j
                    nc.tensor.matmul(
                        w2ps[32 * j:32 * j + 32, 0, :],
                        w2dup_sb[hh:hh + 64, ev, :],
                        h1_sb[hh:hh + 64, :],
                        tile_position=(hh, 32 * j),
                    )
                lg = work.tile([33, S], F32, name="lg", tag="lg")
                nc.scalar.activation(
                    lg[:], w2ps[0:33, 0, :], AF.Identity,
                    bias=b2rep_sb[0:33, ev:ev + 1],
                )
                nc.sync.dma_start(
                    out=out_d[ev, bb:bb + 2, :],
                    in_=lg[0:33:32, :],
                )

            # ---- software-pipelined main loop ----
            project_v(0)
            for eb0 in range(2):
                ps0 = proj_qk_mm(eb0)
                proj_qk_drain(eb0, ps0)
            NEB = EV * B
            for it in range(NEB + 2):
                emit_iteration(it)
                if it >= 3 and (it - 3) % 2 == 0:
                    mlp_tail((it - 3) // 2)
                if it + 2 < NEB:
                    psn = proj_qk_mm(it + 2)
                    proj_qk_drain(it + 2, psn)
                if it < B - 1:
                    project_v(it + 1)

    nc.compile()
    return nc


def _prep_inputs(lstm_features, Wqkv, bqkv, Wo, bo, W1, b1, W2, b2):
    """Host-side per-core input prep (numpy, fp32 -> bf16 where PE-facing)."""
    bf = ml_dtypes.bfloat16
    x = np.asarray(lstm_features, np.float32).reshape(T, D)
    xT = np.ascontiguousarray(x.T).astype(bf)
    scale = 1.0 / np.sqrt(np.float32(Dh))

    in_maps = []
    for c in range(NCORES):
        evs = [2 * c, 2 * c + 1]
        wqkT = np.zeros((D, EV, 2, D), np.float32)
        bqk = np.zeros((D, EV, 2), np.float32)
        wvT = np.zeros((D, EV, D), np.float32)
        w1pT = np.zeros((D, EV, 2, H2), np.float32)
        c1b2 = np.zeros((D, EV), np.float32)
        w2dup = np.zeros((D, EV, 32), np.float32)
        b2rep = np.zeros((D, EV), np.float32)
        for i, e in enumerate(evs):
            Wq = Wqkv[e, 0:D, :] * scale
            Wk = Wqkv[e, D:2 * D, :]
            Wv = Wqkv[e, 2 * D:3 * D, :]
            wqkT[:, i, 0, :] = Wq.T
            wqkT[:, i, 1, :] = Wk.T
            wvT[:, i, :] = Wv.T
            bqk[:, i, 0] = bqkv[e, 0:D] * scale
            bqk[:, i, 1] = bqkv[e, D:2 * D]
            bv = bqkv[e, 2 * D:3 * D]
            bo_eff = Wo[e] @ bv + bo[e]
            W1p = W1[e] @ Wo[e]           # [H2, D]
            # permuted/zero-padded for the packed ctx layout:
            # ctx half g rows 0:32 = head 2g, rows 64:96 = head 2g+1
            for g in range(2):
                w1pT[0:32, i, g, :] = W1p[:, 64 * g:64 * g + 32].T
                w1pT[64:96, i, g, :] = W1p[:, 64 * g + 32:64 * g + 64].T
            c1 = W1[e] @ bo_eff + b1[e]   # [H2]
            c1b2[0:H2, i] = c1
            c1b2[H2:D, i] = c1
            w2dup[0:H2, i, 0] = W2[e, 0, :]
            w2dup[H2:D, i, 0] = W2[e, 0, :]
            b2rep[:, i] = b2[e, 0]
        in_maps.append({
            "xT": xT,
            "wqkT": wqkT.astype(bf),
            "wvT": wvT.astype(bf),
            "bqk": bqk,
            "w1pT": w1pT.astype(bf),
            "c1b2": c1b2,
            "w2dup": w2dup.astype(bf),
            "b2rep": b2rep,
        })
    return in_maps


def kernel(lstm_features, Wqkv, bqkv, Wo, bo, W1, b1, W2, b2, _trace=False):
    global _CACHED_NC
    args = [np.asarray(a, np.float32) for a in
            (lstm_features, Wqkv, bqkv, Wo, bo, W1, b1, W2, b2)]
    in_maps = _prep_inputs(*args)
    if _CACHED_NC is None:
        _CACHED_NC = build_nc()
    res = run_bass_kernel_spmd(
        _CACHED_NC, in_maps, list(range(NCORES)), trace=_trace
    )
    logits = np.concatenate(
        [np.asarray(res.results[c]["out"], np.float32) for c in range(NCORES)],
        axis=0,
    )  # [16, 8, 512]
    out = np.ascontiguousarray(logits.transpose(1, 2, 0))  # [B, S, E]
    if _trace:
        return out, res
    return out
